# revision 11
# baseline (speedup 1.0000x reference)
"""Causal self-attention (B=4, T=2048, C=2048, H=16, fp32) on 8 Trainium2 cores.

Sharding: core c = (batch b=c//2, head-group g=c%2); each core computes 8 heads
of one batch element (tensor-parallel over heads, data-parallel over batch).
The output projection is computed per-group against the matching w_proj row
slice; the two per-batch partials are summed on the host.

All matmuls run as float32r (TF32-like, 1 cycle/row at free-dim >= 256,
~1e-4 relative error). Softmax is computed without max-subtraction (scores
are O(5), exp is safe in fp32) on the transposed score layout S^T[k, q], so
Q/K never need transposing; the causal upper triangle is skipped at block
granularity and diagonal blocks are masked with precomputed 0/1 tiles.
Per-row softmax denominators are built with DVE adds over k-tiles plus an
all-ones matmul that broadcasts the partition-sum to all partitions.
"""

import os
from contextlib import ExitStack

import numpy as np

import jax

jax.config.update("jax_compilation_cache_dir", "/tmp/jax_bass_cache")
jax.config.update("jax_persistent_cache_min_compile_time_secs", 0.0)

import concourse.bass as bass  # noqa: E402
import concourse.mybir as mybir  # noqa: E402
import concourse.tile as tile  # noqa: E402
from concourse import bacc, bass_utils  # noqa: E402
from concourse.masks import make_identity  # noqa: E402

F32 = mybir.dt.float32
F32R = mybir.dt.float32r

B, T, C = 4, 2048, 2048
H, D = 16, 128
HG = 8              # heads per core
HD = HG * D         # 1024 per-core head features
NB = 128            # partition block
TB = T // NB        # 16 token blocks
CB = C // NB        # 16 contraction tiles over C
HB = HD // NB       # 8 feature blocks per projection
CH = 512            # free-dim chunk (one PSUM bank of fp32)
QC = T // CH        # 4 q-chunks
SCALE = 1.0 / float(np.sqrt(D))


def _build():
    nc = bacc.Bacc("TRN2", target_bir_lowering=False, debug=False, num_devices=8)

    x = nc.dram_tensor("x", [T, C], F32R, kind="ExternalInput").ap()
    wq = nc.dram_tensor("wq", [C, HD], F32R, kind="ExternalInput").ap()
    wk = nc.dram_tensor("wk", [C, HD], F32R, kind="ExternalInput").ap()
    wv = nc.dram_tensor("wv", [C, HD], F32R, kind="ExternalInput").ap()
    wp = nc.dram_tensor("wp", [HD, C], F32R, kind="ExternalInput").ap()
    bq = nc.dram_tensor("bq", [HD], F32, kind="ExternalInput").ap()
    bk = nc.dram_tensor("bk", [HD], F32, kind="ExternalInput").ap()
    bv = nc.dram_tensor("bv", [HD], F32, kind="ExternalInput").ap()
    bp = nc.dram_tensor("bp", [C], F32, kind="ExternalInput").ap()
    out = nc.dram_tensor("out", [T, C], F32, kind="ExternalOutput").ap()

    with tile.TileContext(nc) as tc, ExitStack() as ctx:
        # ---- global pools -------------------------------------------------
        dram = ctx.enter_context(tc.tile_pool(name="dram", bufs=1, space="DRAM"))
        qt_sc = [dram.tile([NB, T], F32R, tag=f"qt_sc{h}", name=f"qt_sc{h}")
                 for h in range(HG)]
        kt_sc = [dram.tile([NB, T], F32R, tag=f"kt_sc{h}", name=f"kt_sc{h}")
                 for h in range(HG)]
        v_sc = dram.tile([T, HD], F32R, tag="v_sc")

        const = ctx.enter_context(tc.tile_pool(name="const", bufs=1))
        psA = ctx.enter_context(tc.tile_pool(name="psA", bufs=5, space="PSUM"))
        psB = ctx.enter_context(tc.tile_pool(name="psB", bufs=3, space="PSUM"))

        idn0 = const.tile([NB, NB], F32, tag="idn0")
        make_identity(nc, idn0[:])
        idn = const.tile([NB, NB], F32R, tag="idn")
        nc.vector.tensor_copy(idn[:], idn0[:])
        ones = const.tile([NB, NB], F32R, tag="ones")
        nc.vector.tensor_scalar(ones[:], idn0[:], 0.0, 1.0,
                                mybir.AluOpType.mult, mybir.AluOpType.add)

        # per-head-block bias columns [128, 1]
        bq_t = const.tile([NB, HB], F32, tag="bq")
        nc.sync.dma_start(bq_t[:], bq.rearrange("(h p) -> p h", p=NB))
        bk_t = const.tile([NB, HB], F32, tag="bk")
        nc.sync.dma_start(bk_t[:], bk.rearrange("(h p) -> p h", p=NB))
        bv_row = const.tile([1, HD], F32, tag="bv_row")
        nc.sync.dma_start(bv_row[:], bv[None, :])
        bv_bc = const.tile([NB, HD], F32, tag="bv_bc")
        nc.gpsimd.partition_broadcast(bv_bc[:], bv_row[:])

        # head-0 attention inputs live in an early pool so their loads (issued
        # on the ACT HWDGE queue right after head 0's projections) overlap the
        # rest of the QK phase
        h0_pool = ctx.enter_context(tc.tile_pool(name="h0", bufs=1))
        qt_h0 = h0_pool.tile([NB, T], F32R, tag="qt0")
        kt_h0 = h0_pool.tile([NB, T], F32R, tag="kt0")
        v_h0 = h0_pool.tile([NB, TB, NB], F32R, tag="v0")

        # ---- phases 0 + 1: transpose x, project QKV -----------------------
        with ExitStack() as sa:
            xT_pool = sa.enter_context(tc.tile_pool(name="xT", bufs=1))
            xT_all = xT_pool.tile([NB, CB * T], F32R, tag="xT", name="xT_all")

            def xTs(ci, lo, n):
                return xT_all[:, ci * T + lo: ci * T + lo + n]

            with ExitStack() as s0:
                xr_pool = s0.enter_context(tc.tile_pool(name="xr", bufs=3))
                for tb in range(TB):
                    xr = xr_pool.tile([NB, C], F32R, tag="xr")
                    nc.sync.dma_start(xr[:], x[tb * NB:(tb + 1) * NB, :])
                    for cq in range(CB // 4):
                        tp = psA.tile([NB, 4 * NB], F32R, tag="acc")
                        for i in range(4):
                            ci = 4 * cq + i
                            nc.tensor.transpose(
                                tp[:, i * NB:(i + 1) * NB],
                                xr[:, ci * NB:(ci + 1) * NB], idn[:])
                        dst = xT_all[:, 4 * cq * T + tb * NB:]
                        dst = bass.AP(dst.tensor, dst.offset,
                                      [dst.ap[0], [T, 4], [1, NB]])
                        eng = nc.vector if cq % 2 == 0 else nc.scalar
                        if cq % 2 == 0:
                            nc.vector.tensor_copy(dst, tp[:].rearrange("p (f n) -> p f n", f=4))
                        else:
                            nc.scalar.copy(dst, tp[:].rearrange("p (f n) -> p f n", f=4))

            # V first: [T, feat] orientation, streamed in 256-col quarters
            with ExitStack() as s1v:
                wv_pool = s1v.enter_context(tc.tile_pool(name="wvp", bufs=2))
                vst_pool = s1v.enter_context(tc.tile_pool(name="vst", bufs=4))
                VQ = 256
                for quar in range(HD // VQ):
                    wvc = wv_pool.tile([NB, CB, VQ], F32R, tag="wv")
                    nc.sync.dma_start(
                        wvc[:],
                        wv[:, quar * VQ:(quar + 1) * VQ]
                        .rearrange("(n p) f -> p n f", p=NB))
                    for tb in range(TB):
                        ps = psB.tile([NB, VQ], F32, tag="rot")
                        for ci in range(CB):
                            nc.tensor.matmul(
                                ps[:], xTs(ci, tb * NB, NB),
                                wvc[:, ci, :],
                                start=(ci == 0), stop=(ci == CB - 1))
                        st = vst_pool.tile([NB, VQ], F32R, tag="st")
                        nc.vector.tensor_tensor(
                            st[:], ps[:], bv_bc[:, quar * VQ:(quar + 1) * VQ],
                            mybir.AluOpType.add)
                        nc.sync.dma_start(
                            v_sc[tb * NB:(tb + 1) * NB,
                                 quar * VQ:(quar + 1) * VQ], st[:])

            # QT / KT: [feat, T] orientation, head-major so attention can start
            with ExitStack() as s1:
                wcol_pool = s1.enter_context(tc.tile_pool(name="wcol", bufs=2))
                stage_pool = s1.enter_context(tc.tile_pool(name="qkst", bufs=4))
                for fb in range(HB):
                    for kind in range(2):
                        w_in, b_t = (wq, bq_t) if kind == 0 else (wk, bk_t)
                        o_sc = qt_sc[fb] if kind == 0 else kt_sc[fb]
                        wcol = wcol_pool.tile([NB, CB, NB], F32R, tag="w")
                        nc.sync.dma_start(
                            wcol[:],
                            w_in[:, fb * NB:(fb + 1) * NB]
                            .rearrange("(n p) f -> p n f", p=NB))
                        pss = [psA.tile([NB, CH], F32, tag="acc", name=f"qkps{fb}_{j}")
                               for j in range(QC)]
                        for ci in range(CB):
                            for j in range(QC):
                                nc.tensor.matmul(
                                    pss[j][:], wcol[:, ci, :],
                                    xTs(ci, j * CH, CH),
                                    start=(ci == 0), stop=(ci == CB - 1))
                        for j in range(QC):
                            st = stage_pool.tile([NB, CH], F32R, tag="st")
                            nc.vector.tensor_scalar_add(
                                st[:], pss[j][:], b_t[:, fb:fb + 1])
                            nc.sync.dma_start(
                                o_sc[:, j * CH:(j + 1) * CH], st[:])
                    if fb == 0:
                        nc.scalar.dma_start(qt_h0[:], qt_sc[0][:, :])
                        nc.scalar.dma_start(kt_h0[:], kt_sc[0][:, :])
                        nc.scalar.dma_start(
                            v_h0[:],
                            v_sc[:, 0:NB].rearrange("(n p) d -> p n d", p=NB))

        # ---- phase 2: per-head causal attention ---------------------------
        with ExitStack() as syt:
            yt_pool = syt.enter_context(tc.tile_pool(name="yT", bufs=1))
            yT = [yt_pool.tile([NB, T], F32R, tag=f"yT{h}", name=f"yT{h}")
                  for h in range(HG)]

            with ExitStack() as s2:
                mpool = s2.enter_context(tc.tile_pool(name="masks", bufs=1))
                masks = []
                for m in range(4):
                    mk = mpool.tile([NB, CH], F32, tag=f"mask{m}")
                    nc.gpsimd.memset(mk[:], 1.0)
                    nc.gpsimd.affine_select(
                        out=mk[:], in_=mk[:],
                        compare_op=mybir.AluOpType.is_ge,
                        fill=0.0, base=-(m * NB),
                        pattern=[[1, CH]], channel_multiplier=-1)
                    masks.append(mk)

                qk_pool = s2.enter_context(tc.tile_pool(name="qk", bufs=2))
                v_pool = s2.enter_context(tc.tile_pool(name="vh", bufs=2))
                pt_pool = s2.enter_context(tc.tile_pool(name="pt", bufs=8))
                rcp_pool = s2.enter_context(tc.tile_pool(name="rcp", bufs=2))

                for h in range(HG):
                    if h == 0:
                        qt_h, kt_h, v_h = qt_h0, kt_h0, v_h0
                    else:
                        qt_h = qk_pool.tile([NB, T], F32R, tag="q")
                        nc.scalar.dma_start(qt_h[:], qt_sc[h][:, :])
                        kt_h = qk_pool.tile([NB, T], F32R, tag="k")
                        nc.scalar.dma_start(kt_h[:], kt_sc[h][:, :])
                        v_h = v_pool.tile([NB, TB, NB], F32R, tag="v")
                        nc.scalar.dma_start(
                            v_h[:],
                            v_sc[:, h * NB:(h + 1) * NB]
                            .rearrange("(n p) d -> p n d", p=NB))

                    # two passes of two q-chunks each: PSUM holds ot+sums per
                    # chunk; row-sums accumulate on the PE via the ones matmul
                    for hf in range(2):
                        js = (2 * hf, 2 * hf + 1)
                        ot = {j: psA.tile([NB, CH], F32, tag="acc",
                                          name=f"ot{h}_{j}") for j in js}
                        sums = {j: psA.tile([NB, CH], F32, tag="acc",
                                            name=f"sm{h}_{j}") for j in js}
                        for ki in range(4 * js[-1] + 4):
                            for j in js:
                                if j < ki // QC:
                                    continue
                                st = psB.tile([NB, CH], F32, tag="rot")
                                nc.tensor.matmul(
                                    st[:], kt_h[:, ki * NB:(ki + 1) * NB],
                                    qt_h[:, j * CH:(j + 1) * CH],
                                    start=True, stop=True)
                                pt = pt_pool.tile([NB, CH], F32R, tag="p")
                                nc.scalar.activation(
                                    pt[:], st[:],
                                    mybir.ActivationFunctionType.Exp, scale=SCALE)
                                doff = ki * NB - j * CH
                                if 0 <= doff:
                                    nc.vector.tensor_mul(
                                        pt[:], pt[:], masks[doff // NB][:])
                                last = (ki == 4 * j + 3)
                                nc.tensor.matmul(
                                    ot[j][:], v_h[:, ki, :], pt[:],
                                    start=(ki == 0), stop=last)
                                nc.tensor.matmul(
                                    sums[j][:], ones[:], pt[:],
                                    start=(ki == 0), stop=last)
                        for j in js:
                            rc = rcp_pool.tile([NB, CH], F32, tag="r")
                            nc.vector.reciprocal(rc[:], sums[j][:])
                            nc.vector.tensor_mul(
                                yT[h][:, j * CH:(j + 1) * CH], ot[j][:], rc[:])

            # ---- phase 3: output projection ------------------------------
            with ExitStack() as s3:
                wp_pool = s3.enter_context(tc.tile_pool(name="wp", bufs=1))
                ost_pool = s3.enter_context(tc.tile_pool(name="ost", bufs=4))
                bconst = s3.enter_context(tc.tile_pool(name="bp", bufs=1))
                bp_row = bconst.tile([1, C], F32, tag="bp_row")
                nc.sync.dma_start(bp_row[:], bp[None, :])
                bp_bc = bconst.tile([NB, C], F32, tag="bp_bc")
                nc.gpsimd.partition_broadcast(bp_bc[:], bp_row[:])

                wps = []
                for hb in range(HB):
                    wpt = wp_pool.tile([NB, C], F32R, tag=f"wp{hb}")
                    nc.sync.dma_start(wpt[:], wp[hb * NB:(hb + 1) * NB, :])
                    wps.append(wpt)

                for tb in range(TB):
                    pss = [psA.tile([NB, CH], F32, tag="acc", name=f"pjps{tb}_{j}")
                           for j in range(QC)]
                    for hb in range(HB):
                        for j in range(QC):
                            nc.tensor.matmul(
                                pss[j][:], yT[hb][:, tb * NB:(tb + 1) * NB],
                                wps[hb][:, j * CH:(j + 1) * CH],
                                start=(hb == 0), stop=(hb == HB - 1))
                    for j in range(QC):
                        st = ost_pool.tile([NB, CH], F32, tag="o")
                        nc.vector.tensor_tensor(
                            st[:], pss[j][:], bp_bc[:, j * CH:(j + 1) * CH],
                            mybir.AluOpType.add)
                        nc.sync.dma_start(
                            out[tb * NB:(tb + 1) * NB,
                                j * CH:(j + 1) * CH], st[:])

    nc.compile()
    return nc


_NC = None


def _get_nc():
    global _NC
    if _NC is None:
        _NC = _build()
    return _NC


def _in_maps(x, w_attn, b_attn, w_proj, b_proj):
    x = np.asarray(x, dtype=np.float32)
    w_attn = np.asarray(w_attn, dtype=np.float32)
    b_attn = np.asarray(b_attn, dtype=np.float32)
    w_proj = np.asarray(w_proj, dtype=np.float32)
    b_proj = np.asarray(b_proj, dtype=np.float32)
    zeros_c = np.zeros((C,), dtype=np.float32)
    maps = []
    for core in range(8):
        b, g = core // 2, core % 2
        f0 = g * HD
        maps.append({
            "x": np.ascontiguousarray(x[b]),
            "wq": np.ascontiguousarray(w_attn[:, f0:f0 + HD]),
            "wk": np.ascontiguousarray(w_attn[:, C + f0:C + f0 + HD]),
            "wv": np.ascontiguousarray(w_attn[:, 2 * C + f0:2 * C + f0 + HD]),
            "wp": np.ascontiguousarray(w_proj[f0:f0 + HD, :]),
            "bq": np.ascontiguousarray(b_attn[f0:f0 + HD]),
            "bk": np.ascontiguousarray(b_attn[C + f0:C + f0 + HD]),
            "bv": np.ascontiguousarray(b_attn[2 * C + f0:2 * C + f0 + HD]),
            "bp": (b_proj if g == 0 else zeros_c),
        })
    return maps


def _combine(results):
    outs = [results[c]["out"] for c in range(8)]
    return np.stack([outs[2 * b] + outs[2 * b + 1] for b in range(B)])


def kernel(x, w_attn, b_attn, w_proj, b_proj):
    nc = _get_nc()
    maps = _in_maps(x, w_attn, b_attn, w_proj, b_proj)
    res = bass_utils.run_bass_kernel_spmd(nc, maps, core_ids=list(range(8)))
    return _combine(res.results)


def run_traced(x, w_attn, b_attn, w_proj, b_proj):
    """Like kernel(), but also returns BassKernelResults with HW trace info."""
    nc = _get_nc()
    maps = _in_maps(x, w_attn, b_attn, w_proj, b_proj)
    res = bass_utils.run_bass_kernel_spmd(
        nc, maps, core_ids=list(range(8)), trace=True)
    return _combine(res.results), res


# revision 12
# speedup vs baseline: 1.2013x; 1.2013x over previous
"""Causal self-attention (B=4, T=2048, C=2048, H=16, fp32) on 8 Trainium2 cores.

Sharding: core c = (batch b=c//2, head-group g=c%2); each core computes 8 heads
of one batch element (tensor-parallel over heads, data-parallel over batch).
The output projection is computed per-group against the matching w_proj row
slice; the two per-batch partials are summed on the host.

All matmuls run as float32r (TF32-like, 1 cycle/row at free-dim >= 256,
~1e-4 relative error). Softmax is computed without max-subtraction (scores
are O(5), exp is safe in fp32) on the transposed score layout S^T[k, q], so
Q/K never need transposing; the causal upper triangle is skipped at block
granularity and diagonal blocks are masked with precomputed 0/1 tiles.
Per-row softmax denominators accumulate on the PE via an all-ones matmul
into PSUM (broadcasting the partition-sum to all partitions); attention
runs two q-chunks per pass so ot+sums accumulators fit in PSUM.
"""

from contextlib import ExitStack

import numpy as np

import jax

jax.config.update("jax_compilation_cache_dir", "/tmp/jax_bass_cache")
jax.config.update("jax_persistent_cache_min_compile_time_secs", 0.0)

import concourse.bass as bass  # noqa: E402
import concourse.mybir as mybir  # noqa: E402
import concourse.tile as tile  # noqa: E402
from concourse import bacc, bass_utils  # noqa: E402
from concourse.masks import make_identity  # noqa: E402

F32 = mybir.dt.float32
F32R = mybir.dt.float32r

B, T, C = 4, 2048, 2048
H, D = 16, 128
HG = 8              # heads per core
HD = HG * D         # 1024 per-core head features
NB = 128            # partition block
TB = T // NB        # 16 token blocks
CB = C // NB        # 16 contraction tiles over C
HB = HD // NB       # 8 feature blocks per projection
CH = 512            # free-dim chunk (one PSUM bank of fp32)
QC = T // CH        # 4 q-chunks
SCALE = 1.0 / float(np.sqrt(D))


def _build():
    nc = bacc.Bacc("TRN2", target_bir_lowering=False, debug=False, num_devices=8)

    x = nc.dram_tensor("x", [T, C], F32R, kind="ExternalInput").ap()
    wq = nc.dram_tensor("wq", [C, HD], F32R, kind="ExternalInput").ap()
    wk = nc.dram_tensor("wk", [C, HD], F32R, kind="ExternalInput").ap()
    wv = nc.dram_tensor("wv", [C, HD], F32R, kind="ExternalInput").ap()
    wp = nc.dram_tensor("wp", [HD, C], F32R, kind="ExternalInput").ap()
    bq = nc.dram_tensor("bq", [HD], F32, kind="ExternalInput").ap()
    bk = nc.dram_tensor("bk", [HD], F32, kind="ExternalInput").ap()
    bv = nc.dram_tensor("bv", [HD], F32, kind="ExternalInput").ap()
    bp = nc.dram_tensor("bp", [C], F32, kind="ExternalInput").ap()
    out = nc.dram_tensor("out", [T, C], F32, kind="ExternalOutput").ap()

    with tile.TileContext(nc) as tc, ExitStack() as ctx:
        # ---- global pools -------------------------------------------------
        dram = ctx.enter_context(tc.tile_pool(name="dram", bufs=1, space="DRAM"))
        qt_sc = [dram.tile([NB, T], F32R, tag=f"qt_sc{h}", name=f"qt_sc{h}")
                 for h in range(HG)]
        kt_sc = [dram.tile([NB, T], F32R, tag=f"kt_sc{h}", name=f"kt_sc{h}")
                 for h in range(HG)]
        v_sc = dram.tile([T, HD], F32R, tag="v_sc")

        const = ctx.enter_context(tc.tile_pool(name="const", bufs=1))
        psA = ctx.enter_context(tc.tile_pool(name="psA", bufs=5, space="PSUM"))
        psB = ctx.enter_context(tc.tile_pool(name="psB", bufs=3, space="PSUM"))

        idn0 = const.tile([NB, NB], F32, tag="idn0")
        make_identity(nc, idn0[:])
        idn = const.tile([NB, NB], F32R, tag="idn")
        nc.vector.tensor_copy(idn[:], idn0[:])
        ones = const.tile([NB, NB], F32R, tag="ones")
        nc.vector.tensor_scalar(ones[:], idn0[:], 0.0, 1.0,
                                mybir.AluOpType.mult, mybir.AluOpType.add)

        # per-head-block bias columns [128, 1]
        bq_t = const.tile([NB, HB], F32, tag="bq")
        nc.sync.dma_start(bq_t[:], bq.rearrange("(h p) -> p h", p=NB))
        bk_t = const.tile([NB, HB], F32, tag="bk")
        nc.sync.dma_start(bk_t[:], bk.rearrange("(h p) -> p h", p=NB))
        bv_row = const.tile([1, HD], F32, tag="bv_row")
        nc.sync.dma_start(bv_row[:], bv[None, :])
        bv_bc = const.tile([NB, HD], F32, tag="bv_bc")
        nc.gpsimd.partition_broadcast(bv_bc[:], bv_row[:])

        # head-0 attention inputs live in an early pool so their loads (issued
        # on the ACT HWDGE queue right after head 0's projections) overlap the
        # rest of the QK phase
        h0_pool = ctx.enter_context(tc.tile_pool(name="h0", bufs=1))
        qt_h0 = h0_pool.tile([NB, T], F32R, tag="qt0")
        kt_h0 = h0_pool.tile([NB, T], F32R, tag="kt0")
        v_h0 = h0_pool.tile([NB, TB, NB], F32R, tag="v0")

        # ---- phases 0 + 1: transpose x, project QKV -----------------------
        with ExitStack() as sa:
            xT_pool = sa.enter_context(tc.tile_pool(name="xT", bufs=1))
            xT_all = xT_pool.tile([NB, CB * T], F32R, tag="xT", name="xT_all")

            def xTs(ci, lo, n):
                return xT_all[:, ci * T + lo: ci * T + lo + n]

            with ExitStack() as s0:
                xr_pool = s0.enter_context(tc.tile_pool(name="xr", bufs=3))
                for tb in range(TB):
                    xr = xr_pool.tile([NB, C], F32R, tag="xr")
                    nc.sync.dma_start(xr[:], x[tb * NB:(tb + 1) * NB, :])
                    for cq in range(CB // 4):
                        tp = psA.tile([NB, 4 * NB], F32R, tag="acc")
                        for i in range(4):
                            ci = 4 * cq + i
                            nc.tensor.transpose(
                                tp[:, i * NB:(i + 1) * NB],
                                xr[:, ci * NB:(ci + 1) * NB], idn[:])
                        dst = xT_all[:, 4 * cq * T + tb * NB:]
                        dst = bass.AP(dst.tensor, dst.offset,
                                      [dst.ap[0], [T, 4], [1, NB]])
                        eng = nc.vector if cq % 2 == 0 else nc.scalar
                        if cq % 2 == 0:
                            nc.vector.tensor_copy(dst, tp[:].rearrange("p (f n) -> p f n", f=4))
                        else:
                            nc.scalar.copy(dst, tp[:].rearrange("p (f n) -> p f n", f=4))

            # V first: [T, feat] orientation, streamed in 256-col quarters
            with ExitStack() as s1v:
                wv_pool = s1v.enter_context(tc.tile_pool(name="wvp", bufs=2))
                vst_pool = s1v.enter_context(tc.tile_pool(name="vst", bufs=4))
                VQ = 256
                for quar in range(HD // VQ):
                    wvc = wv_pool.tile([NB, CB, VQ], F32R, tag="wv")
                    nc.sync.dma_start(
                        wvc[:],
                        wv[:, quar * VQ:(quar + 1) * VQ]
                        .rearrange("(n p) f -> p n f", p=NB))
                    for tb in range(TB):
                        ps = psB.tile([NB, VQ], F32, tag="rot")
                        for ci in range(CB):
                            nc.tensor.matmul(
                                ps[:], xTs(ci, tb * NB, NB),
                                wvc[:, ci, :],
                                start=(ci == 0), stop=(ci == CB - 1))
                        st = vst_pool.tile([NB, VQ], F32R, tag="st")
                        nc.vector.tensor_tensor(
                            st[:], ps[:], bv_bc[:, quar * VQ:(quar + 1) * VQ],
                            mybir.AluOpType.add)
                        nc.sync.dma_start(
                            v_sc[tb * NB:(tb + 1) * NB,
                                 quar * VQ:(quar + 1) * VQ], st[:])

            # QT / KT: [feat, T] orientation, head-major so attention can start
            with ExitStack() as s1:
                wcol_pool = s1.enter_context(tc.tile_pool(name="wcol", bufs=2))
                stage_pool = s1.enter_context(tc.tile_pool(name="qkst", bufs=4))
                for fb in range(HB):
                    for kind in range(2):
                        w_in, b_t = (wq, bq_t) if kind == 0 else (wk, bk_t)
                        o_sc = qt_sc[fb] if kind == 0 else kt_sc[fb]
                        wcol = wcol_pool.tile([NB, CB, NB], F32R, tag="w")
                        nc.sync.dma_start(
                            wcol[:],
                            w_in[:, fb * NB:(fb + 1) * NB]
                            .rearrange("(n p) f -> p n f", p=NB))
                        pss = [psA.tile([NB, CH], F32, tag="acc", name=f"qkps{fb}_{j}")
                               for j in range(QC)]
                        for ci in range(CB):
                            for j in range(QC):
                                nc.tensor.matmul(
                                    pss[j][:], wcol[:, ci, :],
                                    xTs(ci, j * CH, CH),
                                    start=(ci == 0), stop=(ci == CB - 1))
                        for j in range(QC):
                            st = stage_pool.tile([NB, CH], F32R, tag="st")
                            nc.vector.tensor_scalar_add(
                                st[:], pss[j][:], b_t[:, fb:fb + 1])
                            nc.sync.dma_start(
                                o_sc[:, j * CH:(j + 1) * CH], st[:])
                    if fb == 0:
                        nc.scalar.dma_start(qt_h0[:], qt_sc[0][:, :])
                        nc.scalar.dma_start(kt_h0[:], kt_sc[0][:, :])
                        nc.scalar.dma_start(
                            v_h0[:],
                            v_sc[:, 0:NB].rearrange("(n p) d -> p n d", p=NB))

        # ---- phase 2: per-head causal attention ---------------------------
        with ExitStack() as syt:
            yt_pool = syt.enter_context(tc.tile_pool(name="yT", bufs=1))
            yT = [yt_pool.tile([NB, T], F32R, tag=f"yT{h}", name=f"yT{h}")
                  for h in range(HG)]

            with ExitStack() as s2:
                mpool = s2.enter_context(tc.tile_pool(name="masks", bufs=1))
                masks = []
                for m in range(4):
                    mk = mpool.tile([NB, CH], F32, tag=f"mask{m}")
                    nc.gpsimd.memset(mk[:], 1.0)
                    nc.gpsimd.affine_select(
                        out=mk[:], in_=mk[:],
                        compare_op=mybir.AluOpType.is_ge,
                        fill=0.0, base=-(m * NB),
                        pattern=[[1, CH]], channel_multiplier=-1)
                    masks.append(mk)

                qk_pool = s2.enter_context(tc.tile_pool(name="qk", bufs=2))
                v_pool = s2.enter_context(tc.tile_pool(name="vh", bufs=2))
                pt_pool = s2.enter_context(tc.tile_pool(name="pt", bufs=8))
                rcp_pool = s2.enter_context(tc.tile_pool(name="rcp", bufs=2))

                for h in range(HG):
                    if h == 0:
                        qt_h, kt_h, v_h = qt_h0, kt_h0, v_h0
                    else:
                        qt_h = qk_pool.tile([NB, T], F32R, tag="q")
                        nc.scalar.dma_start(qt_h[:], qt_sc[h][:, :])
                        kt_h = qk_pool.tile([NB, T], F32R, tag="k")
                        nc.scalar.dma_start(kt_h[:], kt_sc[h][:, :])
                        v_h = v_pool.tile([NB, TB, NB], F32R, tag="v")
                        nc.scalar.dma_start(
                            v_h[:],
                            v_sc[:, h * NB:(h + 1) * NB]
                            .rearrange("(n p) d -> p n d", p=NB))

                    # two passes of two q-chunks each: PSUM holds ot+sums per
                    # chunk; row-sums accumulate on the PE via the ones matmul
                    for hf in range(2):
                        js = (2 * hf, 2 * hf + 1)
                        ot = {j: psA.tile([NB, CH], F32, tag="acc",
                                          name=f"ot{h}_{j}") for j in js}
                        sums = {j: psA.tile([NB, CH], F32, tag="acc",
                                            name=f"sm{h}_{j}") for j in js}
                        for ki in range(4 * js[-1] + 4):
                            for j in js:
                                if j < ki // QC:
                                    continue
                                st = psB.tile([NB, CH], F32, tag="rot")
                                nc.tensor.matmul(
                                    st[:], kt_h[:, ki * NB:(ki + 1) * NB],
                                    qt_h[:, j * CH:(j + 1) * CH],
                                    start=True, stop=True)
                                pt = pt_pool.tile([NB, CH], F32R, tag="p")
                                nc.scalar.activation(
                                    pt[:], st[:],
                                    mybir.ActivationFunctionType.Exp, scale=SCALE)
                                doff = ki * NB - j * CH
                                if 0 <= doff:
                                    nc.vector.tensor_mul(
                                        pt[:], pt[:], masks[doff // NB][:])
                                last = (ki == 4 * j + 3)
                                nc.tensor.matmul(
                                    ot[j][:], v_h[:, ki, :], pt[:],
                                    start=(ki == 0), stop=last)
                                nc.tensor.matmul(
                                    sums[j][:], ones[:], pt[:],
                                    start=(ki == 0), stop=last)
                        for j in js:
                            rc = rcp_pool.tile([NB, CH], F32, tag="r")
                            nc.vector.reciprocal(rc[:], sums[j][:])
                            nc.vector.tensor_mul(
                                yT[h][:, j * CH:(j + 1) * CH], ot[j][:], rc[:])

            # ---- phase 3: output projection ------------------------------
            with ExitStack() as s3:
                wp_pool = s3.enter_context(tc.tile_pool(name="wp", bufs=1))
                ost_pool = s3.enter_context(tc.tile_pool(name="ost", bufs=4))
                bconst = s3.enter_context(tc.tile_pool(name="bp", bufs=1))
                bp_row = bconst.tile([1, C], F32, tag="bp_row")
                nc.sync.dma_start(bp_row[:], bp[None, :])
                bp_bc = bconst.tile([NB, C], F32, tag="bp_bc")
                nc.gpsimd.partition_broadcast(bp_bc[:], bp_row[:])

                wps = []
                for hb in range(HB):
                    wpt = wp_pool.tile([NB, C], F32R, tag=f"wp{hb}")
                    nc.sync.dma_start(wpt[:], wp[hb * NB:(hb + 1) * NB, :])
                    wps.append(wpt)

                for tb in range(TB):
                    pss = [psA.tile([NB, CH], F32, tag="acc", name=f"pjps{tb}_{j}")
                           for j in range(QC)]
                    for hb in range(HB):
                        for j in range(QC):
                            nc.tensor.matmul(
                                pss[j][:], yT[hb][:, tb * NB:(tb + 1) * NB],
                                wps[hb][:, j * CH:(j + 1) * CH],
                                start=(hb == 0), stop=(hb == HB - 1))
                    for j in range(QC):
                        st = ost_pool.tile([NB, CH], F32, tag="o")
                        nc.vector.tensor_tensor(
                            st[:], pss[j][:], bp_bc[:, j * CH:(j + 1) * CH],
                            mybir.AluOpType.add)
                        nc.sync.dma_start(
                            out[tb * NB:(tb + 1) * NB,
                                j * CH:(j + 1) * CH], st[:])

    nc.compile()
    return nc


_NC = None


def _get_nc():
    global _NC
    if _NC is None:
        _NC = _build()
    return _NC


def _in_maps(x, w_attn, b_attn, w_proj, b_proj):
    x = np.asarray(x, dtype=np.float32)
    w_attn = np.asarray(w_attn, dtype=np.float32)
    b_attn = np.asarray(b_attn, dtype=np.float32)
    w_proj = np.asarray(w_proj, dtype=np.float32)
    b_proj = np.asarray(b_proj, dtype=np.float32)
    zeros_c = np.zeros((C,), dtype=np.float32)
    maps = []
    for core in range(8):
        b, g = core // 2, core % 2
        f0 = g * HD
        maps.append({
            "x": np.ascontiguousarray(x[b]),
            "wq": np.ascontiguousarray(w_attn[:, f0:f0 + HD]),
            "wk": np.ascontiguousarray(w_attn[:, C + f0:C + f0 + HD]),
            "wv": np.ascontiguousarray(w_attn[:, 2 * C + f0:2 * C + f0 + HD]),
            "wp": np.ascontiguousarray(w_proj[f0:f0 + HD, :]),
            "bq": np.ascontiguousarray(b_attn[f0:f0 + HD]),
            "bk": np.ascontiguousarray(b_attn[C + f0:C + f0 + HD]),
            "bv": np.ascontiguousarray(b_attn[2 * C + f0:2 * C + f0 + HD]),
            "bp": (b_proj if g == 0 else zeros_c),
        })
    return maps


def _combine(results):
    outs = [results[c]["out"] for c in range(8)]
    return np.stack([outs[2 * b] + outs[2 * b + 1] for b in range(B)])


def kernel(x, w_attn, b_attn, w_proj, b_proj):
    nc = _get_nc()
    maps = _in_maps(x, w_attn, b_attn, w_proj, b_proj)
    res = bass_utils.run_bass_kernel_spmd(nc, maps, core_ids=list(range(8)))
    return _combine(res.results)


def run_traced(x, w_attn, b_attn, w_proj, b_proj):
    """Like kernel(), but also returns BassKernelResults with HW trace info."""
    nc = _get_nc()
    maps = _in_maps(x, w_attn, b_attn, w_proj, b_proj)
    res = bass_utils.run_bass_kernel_spmd(
        nc, maps, core_ids=list(range(8)), trace=True)
    return _combine(res.results), res
